# revision 48
# baseline (speedup 1.0000x reference)
"""ConvLSTM (T=16, B=4, C=32, HID=64, 64x64, 3x3 convs) on 8 Trainium2 cores.

Decomposition: 8 cores = batch(4) x H-halves(2). Each core owns 32 output rows
and recomputes a shrinking halo (rows 0..47-t at step t) so NO inter-core
communication is needed. The bottom-half cores get vertically flipped inputs
(and dy-flipped conv weights) so all 8 cores run the identical SPMD program.

Conv-as-matmul with dense tap packing (7 matmuls per 128-out-channel group per
512-pixel chunk = the partition-packing floor for 864 contraction rows):
    X1 = x taps (0,0)(0,1)(0,2)(1,0)   [4x32 rows, shifts 0,1,2,66]
    X2 = x taps (1,1)(1,2)(2,0)(2,1)   [shifts 67,68,132,133]
    HA @ dy, dy=0,1,2 = h taps (dy,0)(dy,1)  [2x64 rows, col flavors -1,0]
    HB2 = h taps (0,2)(2,2)            [col flavor +1 at row offsets 0,2]
    CMB = x tap (2,2) + h tap (1,2)    [96 rows]

This revision restructures the dataflow around measured per-instruction
fixed costs (DVE ~151 cyc, ACT ~352 cyc, PE p-state ramp 0.65->2.4 GHz,
~100 ns semaphore hops) and DMA packet-rate limits:
  * h = o*tanh(c) is written by the DVE straight into the padded hA image
    (parts 64:128 = the dx=1 column flavor; pad columns stay zero from the
    init memset). Every other tap flavor (dx=0 via hA[0:64], dx=2 via hB2
    and cmb) is then ONE contiguous whole-row SBUF copy at a flat offset:
    64 x ~1KB descriptors instead of 512 x 128B - this removed the ~200k
    DMA-packet storm that dominated the original kernel.
  * PSUM gate grouping [g; i] / [f; o]: sigma over merged [f;o] is one
    128-partition ACT; tanh(g) needs no partition-base shift; sigma(i) and
    tanh(c) use ACT's partition-base shifting so every DVE product pairs
    at a single partition base (TensorTensor requires equal bases).
  * 8-row chunks with 1-bank PSUM tiles, bufs=4: four chunks in flight
    keep the PE streaming (p-state at full clock); x-only matmuls are
    emitted before h matmuls per chunk for dependency-free boundary work.
  * All gating tensors and c in bf16 (DVE 2x mode).
  * DMA queues: sync = xb loads + dx0/dx2 flavor copies + output;
    gpsimd = xa loads + hB2[64:128]/cmb copies; scalar = ACTs only.
  * Output is written as padded 66-wide rows (contiguous) in bf16; the
    host strips pads and widens.
"""
import sys
import os

for _p in ("/opt/trn_rl_repo", "/root/.axon_site"):
    if _p not in sys.path and os.path.isdir(_p):
        sys.path.append(_p)

import numpy as np

T, B, C, H, W, HID = 16, 4, 32, 64, 64, 64
HP, WP = 49, 66          # padded per-core image: 48 data rows + 1 top pad, 64+2 cols
FLAT = HP * WP           # 3234
XLEN = 3100              # per-block x DMA length (covers max read f=3099)

X_TAPS_A = [(0, 0), (0, 1), (0, 2), (1, 0)]
X_TAPS_B = [(1, 1), (1, 2), (2, 0), (2, 1)]

_CACHE = {}


def _build_program():
    import concourse.mybir as mybir
    import concourse.tile as tile
    from concourse import bacc

    f32 = mybir.dt.float32
    bf16 = mybir.dt.bfloat16

    nc = bacc.Bacc("TRN2", target_bir_lowering=False, debug=False, num_devices=8)

    xp_d = nc.dram_tensor("xp", [T, C, FLAT], bf16, kind="ExternalInput")
    wx1_d = nc.dram_tensor("wx1", [128, 256], bf16, kind="ExternalInput")
    wx2_d = nc.dram_tensor("wx2", [128, 256], bf16, kind="ExternalInput")
    wa_d = nc.dram_tensor("wa", [128, 768], bf16, kind="ExternalInput")
    wb2_d = nc.dram_tensor("wb2", [128, 256], bf16, kind="ExternalInput")
    wc_d = nc.dram_tensor("wc", [96, 256], bf16, kind="ExternalInput")
    bias_d = nc.dram_tensor("bias", [128, 2], f32, kind="ExternalInput")
    # 2 blocks of 16 padded 66-wide rows per step; host strips the pads
    out_d = nc.dram_tensor("out", [T, HID, 2112], bf16, kind="ExternalOutput")

    Sigmoid = mybir.ActivationFunctionType.Sigmoid
    Tanh = mybir.ActivationFunctionType.Tanh
    mult = mybir.AluOpType.mult
    add = mybir.AluOpType.add

    with tile.TileContext(nc) as tc:
        with tc.tile_pool(name="const", bufs=1) as constp, \
             tc.tile_pool(name="xpool", bufs=3) as xpool, \
             tc.tile_pool(name="hpool", bufs=1) as hpool, \
             tc.tile_pool(name="cpool", bufs=1) as cpool, \
             tc.tile_pool(name="psum", bufs=4, space="PSUM") as psum, \
             tc.tile_pool(name="spool", bufs=3) as spool:

            wx1_s = constp.tile([128, 256], bf16)
            wx2_s = constp.tile([128, 256], bf16)
            wa_s = constp.tile([128, 768], bf16)
            wb2_s = constp.tile([128, 256], bf16)
            wc_s = constp.tile([96, 256], bf16)
            bias_s = constp.tile([128, 2], f32)
            for s_, d_ in [(wx1_s, wx1_d), (wx2_s, wx2_d), (wa_s, wa_d),
                           (wb2_s, wb2_d), (wc_s, wc_d), (bias_s, bias_d)]:
                nc.sync.dma_start(s_[:], d_[:])

            # ping-pong h tiles (shifted partition blocks, see module docstring)
            hA = [hpool.tile([128, FLAT], bf16, tag=f"hA{i}", name=f"hA{i}")
                  for i in range(2)]
            hB2 = [hpool.tile([128, FLAT], bf16, tag=f"hB2{i}", name=f"hB2{i}")
                   for i in range(2)]
            # cmb: parts 0-31 x tap (2,2) [per-step], parts 32-95 h tap (1,2)
            cmb = [hpool.tile([96, FLAT], bf16, tag=f"cmb{i}", name=f"cmb{i}")
                   for i in range(2)]
            for t_ in hA + hB2 + cmb:
                nc.vector.memset(t_[:], 0.0)

            c_s = cpool.tile([64, 47 * 64], bf16)

            def load_x(t):
                # x tiles for step t: TA (4 blocks), TB (4 blocks), cmb
                # x-block, issued one step ahead. Loads shrink with the halo:
                # step t only reads rows 0..R-1 of the padded image.
                xlen = (48 - t) * WP - 2
                xa = xpool.tile([128, FLAT], bf16, tag="xa", name="xa")
                xb = xpool.tile([128, FLAT], bf16, tag="xb", name="xb")
                for b3, (dy, dx) in enumerate(X_TAPS_A):
                    s = dy * WP + dx
                    nc.gpsimd.dma_start(xa[32 * b3:32 * b3 + 32, 0:xlen],
                                        xp_d[t - 1, :, s:s + xlen])
                for b3, (dy, dx) in enumerate(X_TAPS_B):
                    s = dy * WP + dx
                    nc.sync.dma_start(xb[32 * b3:32 * b3 + 32, 0:xlen],
                                      xp_d[t - 1, :, s:s + xlen])
                nc.gpsimd.dma_start(cmb[t % 2][0:32, 0:xlen],
                                    xp_d[t - 1, :, 134:134 + xlen])
                return xa, xb

            xtiles = load_x(1)
            for t in range(1, T + 1):
                R = 48 - t
                xa, xb = xtiles
                if t < T:
                    xtiles = load_x(t + 1)
                hAp, hB2p, cmbp = hA[(t - 1) % 2], hB2[(t - 1) % 2], cmb[t % 2]
                hAc, hB2c, cmbn = hA[t % 2], hB2[t % 2], cmb[(t + 1) % 2]
                xav = xa[:].rearrange("p (y x) -> p y x", x=WP)
                xbv = xb[:].rearrange("p (y x) -> p y x", x=WP)
                hAv = hAp[:].rearrange("p (y x) -> p y x", x=WP)
                hB2v = hB2p[:].rearrange("p (y x) -> p y x", x=WP)
                cmbv = cmbp[:].rearrange("p (y x) -> p y x", x=WP)

                # 8-row chunks, one 1-bank PSUM tile pair each; bufs=4 keeps
                # four chunks in flight for deep PE pipelining
                chunks = [(y0, min(8, R - y0)) for y0 in range(0, R, 8)]
                hw3 = hAc[64:128, :].rearrange("p (y x) -> p y x", x=WP)

                def emit_xmm(y0, nr, psA, psB):
                    for gp, pt in ((0, psA), (1, psB)):
                        gs = gp * 128
                        dst = pt[:, 0:nr * 64]
                        nc.tensor.matmul(dst, wx1_s[:, gs:gs + 128],
                                         xav[:, y0:y0 + nr, 0:64],
                                         start=True, stop=False)
                        nc.tensor.matmul(dst, wx2_s[:, gs:gs + 128],
                                         xbv[:, y0:y0 + nr, 0:64],
                                         start=False, stop=False)

                # one-chunk x-matmul lookahead: the PE reaches each chunk's
                # h-tile wait with the next chunk's x work already banked
                pstiles = {}
                pstiles[0] = (psum.tile([128, 512], f32, tag="psA",
                                        name="psA"),
                              psum.tile([128, 512], f32, tag="psB",
                                        name="psB"))
                emit_xmm(chunks[0][0], chunks[0][1], *pstiles[0])
                for q, (y0, nr) in enumerate(chunks):
                    N = nr * 64
                    psA, psB = pstiles.pop(q)
                    if q + 1 < len(chunks):
                        pstiles[q + 1] = (psum.tile([128, 512], f32,
                                                    tag="psA", name="psA"),
                                          psum.tile([128, 512], f32,
                                                    tag="psB", name="psB"))
                        emit_xmm(chunks[q + 1][0], chunks[q + 1][1],
                                 *pstiles[q + 1])
                    for gp, pt in ((0, psA), (1, psB)):
                        gs = gp * 128
                        dst = pt[:, 0:N]
                        if t > 1:
                            for dy in range(3):
                                nc.tensor.matmul(
                                    dst,
                                    wa_s[:, (dy * 2 + gp) * 128:
                                         (dy * 2 + gp + 1) * 128],
                                    hAv[:, y0 + dy:y0 + dy + nr, 0:64],
                                    start=False, stop=False)
                            nc.tensor.matmul(dst, wb2_s[:, gs:gs + 128],
                                             hB2v[:, y0:y0 + nr, 0:64],
                                             start=False, stop=False)
                            nc.tensor.matmul(dst, wc_s[:, gs:gs + 128],
                                             cmbv[0:96, y0:y0 + nr, 0:64],
                                             start=False, stop=True)
                        else:
                            nc.tensor.matmul(dst, wc_s[0:32, gs:gs + 128],
                                             cmbv[0:32, y0:y0 + nr, 0:64],
                                             start=False, stop=True)

                    # gating: 3 gate ACTs (sigma merged [f;o]; tanh(g) needs
                    # no partition shift with the [g;i] grouping; sigma(i)
                    # shifts down), then the c chain on DVE, tanh(c) shifted
                    # up, and h = o*tanh(c) written straight into the padded
                    # hA image (parts 64:128 = dx=1 col flavor; pad cols
                    # stay zero from init). Every other flavor is then a
                    # CONTIGUOUS whole-row copy of this region: one ~2KB
                    # descriptor per partition instead of 8 x 128B.
                    s01 = spool.tile([128, 512], bf16, tag="s01", name="s01")
                    s23 = spool.tile([128, 512], bf16, tag="s23", name="s23")
                    gt = spool.tile([64, 512], bf16, tag="gt", name="gt")
                    tc_ = spool.tile([128, 512], bf16, tag="tc", name="tc")
                    t1 = spool.tile([64, 512], bf16, tag="t1", name="t1")
                    c_sl = c_s[:, y0 * 64:y0 * 64 + N]
                    # one sigma covers i AND g via tanh(x) = 2*sigmoid(2x)-1
                    # (g weights+bias doubled on the host); the 2s-1 fixup
                    # runs on the DVE, partition-shifting P1->P0
                    nc.scalar.activation(s01[:, :N], psA[:, :N], Sigmoid,
                                         bias=bias_s[:, 0:1])
                    nc.scalar.activation(s23[:, :N], psB[:, :N], Sigmoid,
                                         bias=bias_s[:, 1:2])
                    nc.vector.tensor_scalar(gt[:, :N], s01[64:128, :N],
                                            2.0, -1.0, op0=mult, op1=add)
                    if t == 1:
                        nc.vector.tensor_mul(c_sl, s01[0:64, :N], gt[:, :N])
                    else:
                        nc.vector.tensor_mul(c_sl, s23[0:64, :N], c_sl)
                        nc.vector.tensor_mul(t1[:, :N], s01[0:64, :N],
                                             gt[:, :N])
                        nc.vector.tensor_add(c_sl, c_sl, t1[:, :N])
                    nc.scalar.activation(tc_[64:128, :N], c_sl, Tanh)
                    nc.vector.tensor_mul(
                        hw3[:, y0 + 1:y0 + 1 + nr, 1:65],
                        s23[64:128, :N].rearrange("p (y x) -> p y x", x=64),
                        tc_[64:128, :N].rearrange("p (y x) -> p y x", x=64))
                    d0 = (y0 + 1) * WP
                    L = nr * WP
                    if t < T:
                        # dx=0 flavor: shift by one column
                        nc.sync.dma_start(hAc[0:64, d0:d0 + L],
                                          hAc[64:128, d0 + 1:d0 + 1 + L])
                        # dx=2 flavor at row offsets 0 / 2 / 1
                        nc.sync.dma_start(hB2c[0:64, d0:d0 + L],
                                          hAc[64:128, d0 + 2:d0 + 2 + L])
                        if q == 0:
                            nc.gpsimd.dma_start(
                                hB2c[64:128, 0:L - WP],
                                hAc[64:128, d0 + WP + 2:d0 + 2 + L])
                        else:
                            nc.gpsimd.dma_start(
                                hB2c[64:128, d0 - 2 * WP:d0 - 2 * WP + L],
                                hAc[64:128, d0 + 2:d0 + 2 + L])
                        nc.gpsimd.dma_start(
                            cmbn[32:96, d0 - WP:d0 - WP + L],
                            hAc[64:128, d0 + 2:d0 + 2 + L])
                    if y0 < 32:
                        o0 = y0 * WP
                        nc.sync.dma_start(
                            out_d[t - 1, :, o0:o0 + L],
                            hAc[64:128, d0:d0 + L])

    nc.compile()
    return nc


def _host_prep(x, w_x2h, b_x2h, w_h2h, b_h2h):
    """Build the 8 per-core input maps."""
    import ml_dtypes
    np_bf16 = ml_dtypes.bfloat16

    x = np.ascontiguousarray(np.asarray(x, np.float32))
    w_x2h = np.asarray(w_x2h, np.float32)
    b_x2h = np.asarray(b_x2h, np.float32)
    w_h2h = np.asarray(w_h2h, np.float32)
    b_h2h = np.asarray(b_h2h, np.float32)

    bsum = b_x2h + b_h2h
    # gate-pair grouping: gp0 = [g; i], gp1 = [f; o]
    bias = np.zeros((128, 2), np.float32)
    bias[0:64, 0] = bsum[0:64]
    bias[64:128, 0] = 2.0 * bsum[128:192]
    bias[0:64, 1] = bsum[64:128]
    bias[64:128, 1] = bsum[192:256]

    # packed out-channel index list per gate-pair: gp0 = [i; 2g], gp1 = [f; o]
    oc_idx = {0: np.r_[0:64, 128:192], 1: np.r_[64:128, 192:256]}
    oc_scale = {0: np.r_[np.ones(64), 2.0 * np.ones(64)].astype(np.float32),
                1: np.ones(128, np.float32)}

    in_maps = []
    packed_w = {}
    for parity in range(2):
        wx_f = w_x2h if parity == 0 else w_x2h[:, :, ::-1, :]
        wh_f = w_h2h if parity == 0 else w_h2h[:, :, ::-1, :]
        wx1 = np.zeros((128, 2, 128), np.float32)
        wx2 = np.zeros((128, 2, 128), np.float32)
        wa = np.zeros((128, 3, 2, 128), np.float32)
        wb2 = np.zeros((128, 2, 128), np.float32)
        wc = np.zeros((96, 2, 128), np.float32)
        for gp in range(2):
            oc, sc = oc_idx[gp], oc_scale[gp][None, :]
            for b3, (dy, dx) in enumerate(X_TAPS_A):
                wx1[32 * b3:32 * b3 + 32, gp, :] = wx_f[oc, :, dy, dx].T * sc
            for b3, (dy, dx) in enumerate(X_TAPS_B):
                wx2[32 * b3:32 * b3 + 32, gp, :] = wx_f[oc, :, dy, dx].T * sc
            for dy in range(3):
                for b3 in range(2):
                    # parts 0:64 = dx1 flavor, 64:128 = dx0 (TT writes dx1)
                    wa[64 * b3:64 * b3 + 64, dy, gp, :] = \
                        wh_f[oc, :, dy, 1 - b3].T * sc
            wb2[0:64, gp, :] = wh_f[oc, :, 0, 2].T * sc
            wb2[64:128, gp, :] = wh_f[oc, :, 2, 2].T * sc
            wc[0:32, gp, :] = wx_f[oc, :, 2, 2].T * sc
            wc[32:96, gp, :] = wh_f[oc, :, 1, 2].T * sc
        packed_w[parity] = tuple(
            np.ascontiguousarray(a.reshape(a.shape[0], -1).astype(np_bf16))
            for a in (wx1, wx2, wa, wb2, wc))

    for core in range(8):
        b, parity = core // 2, core % 2
        xv = x[:, b]
        if parity == 1:
            xv = xv[:, :, ::-1, :]
        xp = np.zeros((T, C, HP, WP), np.float32)
        xp[:, :, 1:49, 1:65] = xv[:, :, 0:48, :]
        wx1, wx2, wa, wb2, wc = packed_w[parity]
        in_maps.append({
            "xp": np.ascontiguousarray(xp.reshape(T, C, FLAT).astype(np_bf16)),
            "wx1": wx1, "wx2": wx2, "wa": wa, "wb2": wb2, "wc": wc,
            "bias": bias,
        })
    return in_maps


def kernel(x, w_x2h, b_x2h, w_h2h, b_h2h, _trace=False, _tmpdir=None):
    from concourse.bass_utils import run_bass_kernel_spmd

    if "nc" not in _CACHE:
        _CACHE["nc"] = _build_program()
    nc = _CACHE["nc"]

    in_maps = _host_prep(x, w_x2h, b_x2h, w_h2h, b_h2h)
    kw = {}
    if _trace:
        kw = dict(trace=True, tmpdir=_tmpdir)
    res = run_bass_kernel_spmd(nc, in_maps, core_ids=list(range(8)), **kw)

    full = np.zeros((T, B, HID, H, W), np.float32)
    for core in range(8):
        b, parity = core // 2, core % 2
        out = np.asarray(res.results[core]["out"]).astype(np.float32)
        out = out.reshape(T, HID, 32, 66)[:, :, :, 1:65]
        if parity == 0:
            full[:, b, :, 0:32] = out
        else:
            full[:, b, :, 32:64] = out[:, :, ::-1, :]
    if _trace:
        return full, res
    return full


# revision 49
# speedup vs baseline: 1.1810x; 1.1810x over previous
"""ConvLSTM (T=16, B=4, C=32, HID=64, 64x64, 3x3 convs) on 8 Trainium2 cores.

Decomposition: 8 cores = batch(4) x H-halves(2). Each core owns 32 output rows
and recomputes a shrinking halo (rows 0..47-t at step t) so NO inter-core
communication is needed. The bottom-half cores get vertically flipped inputs
(and dy-flipped conv weights) so all 8 cores run the identical SPMD program.

Conv-as-matmul with dense tap packing (7 matmuls per 128-out-channel group per
512-pixel chunk = the partition-packing floor for 864 contraction rows):
    X1 = x taps (0,0)(0,1)(0,2)(1,0)   [4x32 rows, shifts 0,1,2,66]
    X2 = x taps (1,1)(1,2)(2,0)(2,1)   [shifts 67,68,132,133]
    HA @ dy, dy=0,1,2 = h taps (dy,0)(dy,1)  [2x64 rows, col flavors -1,0]
    HB2 = h taps (0,2)(2,2)            [col flavor +1 at row offsets 0,2]
    CMB = x tap (2,2) + h tap (1,2)    [96 rows]

This revision restructures the dataflow around measured per-instruction
fixed costs (DVE ~151 cyc, ACT ~352 cyc, PE p-state ramp 0.65->2.4 GHz,
~100 ns semaphore hops) and DMA packet-rate limits:
  * h = o*tanh(c) is written by the DVE straight into the padded hA image
    (parts 64:128 = the dx=1 column flavor; pad columns stay zero from the
    init memset). Every other tap flavor (dx=0 via hA[0:64], dx=2 via hB2
    and cmb) is then ONE contiguous whole-row SBUF copy at a flat offset:
    64 x ~1KB descriptors instead of 512 x 128B - this removed the ~200k
    DMA-packet storm that dominated the original kernel.
  * PSUM gate grouping [g; i] / [f; o]: sigma over merged [f;o] is one
    128-partition ACT; tanh(g) needs no partition-base shift; sigma(i) and
    tanh(c) use ACT's partition-base shifting so every DVE product pairs
    at a single partition base (TensorTensor requires equal bases).
  * 8-row chunks with 1-bank PSUM tiles, bufs=4: four chunks in flight
    keep the PE streaming (p-state at full clock); x-only matmuls are
    emitted before h matmuls per chunk for dependency-free boundary work.
  * All gating tensors and c in bf16 (DVE 2x mode).
  * DMA queues: sync = xb loads + dx0/dx2 flavor copies + output;
    gpsimd = xa loads + hB2[64:128]/cmb copies; scalar = ACTs only.
  * Output is written as padded 66-wide rows (contiguous) in bf16; the
    host strips pads and widens.
"""
import sys
import os

for _p in ("/opt/trn_rl_repo", "/root/.axon_site"):
    if _p not in sys.path and os.path.isdir(_p):
        sys.path.append(_p)

import numpy as np

T, B, C, H, W, HID = 16, 4, 32, 64, 64, 64
HP, WP = 49, 66          # padded per-core image: 48 data rows + 1 top pad, 64+2 cols
FLAT = HP * WP           # 3234
XLEN = 3100              # per-block x DMA length (covers max read f=3099)

X_TAPS_A = [(0, 0), (0, 1), (0, 2), (1, 0)]
X_TAPS_B = [(1, 1), (1, 2), (2, 0), (2, 1)]

_CACHE = {}


def _build_program():
    import concourse.mybir as mybir
    import concourse.tile as tile
    from concourse import bacc

    f32 = mybir.dt.float32
    bf16 = mybir.dt.bfloat16

    nc = bacc.Bacc("TRN2", target_bir_lowering=False, debug=False, num_devices=8)

    xp_d = nc.dram_tensor("xp", [T, C, FLAT], bf16, kind="ExternalInput")
    wx1_d = nc.dram_tensor("wx1", [128, 256], bf16, kind="ExternalInput")
    wx2_d = nc.dram_tensor("wx2", [128, 256], bf16, kind="ExternalInput")
    wa_d = nc.dram_tensor("wa", [128, 768], bf16, kind="ExternalInput")
    wb2_d = nc.dram_tensor("wb2", [128, 256], bf16, kind="ExternalInput")
    wc_d = nc.dram_tensor("wc", [96, 256], bf16, kind="ExternalInput")
    bias_d = nc.dram_tensor("bias", [128, 2], f32, kind="ExternalInput")
    # 2 blocks of 16 padded 66-wide rows per step; host strips the pads
    out_d = nc.dram_tensor("out", [T, HID, 2112], bf16, kind="ExternalOutput")

    Sigmoid = mybir.ActivationFunctionType.Sigmoid
    Tanh = mybir.ActivationFunctionType.Tanh
    mult = mybir.AluOpType.mult
    add = mybir.AluOpType.add

    with tile.TileContext(nc) as tc:
        with tc.tile_pool(name="const", bufs=1) as constp, \
             tc.tile_pool(name="xpool", bufs=3) as xpool, \
             tc.tile_pool(name="hpool", bufs=1) as hpool, \
             tc.tile_pool(name="cpool", bufs=1) as cpool, \
             tc.tile_pool(name="psum", bufs=4, space="PSUM") as psum, \
             tc.tile_pool(name="spool", bufs=3) as spool:

            wx1_s = constp.tile([128, 256], bf16)
            wx2_s = constp.tile([128, 256], bf16)
            wa_s = constp.tile([128, 768], bf16)
            wb2_s = constp.tile([128, 256], bf16)
            wc_s = constp.tile([96, 256], bf16)
            bias_s = constp.tile([128, 2], f32)
            for s_, d_ in [(wx1_s, wx1_d), (wx2_s, wx2_d), (wa_s, wa_d),
                           (wb2_s, wb2_d), (wc_s, wc_d), (bias_s, bias_d)]:
                nc.sync.dma_start(s_[:], d_[:])

            # ping-pong h tiles (shifted partition blocks, see module docstring)
            hA = [hpool.tile([128, FLAT], bf16, tag=f"hA{i}", name=f"hA{i}")
                  for i in range(2)]
            hB2 = [hpool.tile([128, FLAT], bf16, tag=f"hB2{i}", name=f"hB2{i}")
                   for i in range(2)]
            # cmb: parts 0-31 x tap (2,2) [per-step], parts 32-95 h tap (1,2)
            cmb = [hpool.tile([96, FLAT], bf16, tag=f"cmb{i}", name=f"cmb{i}")
                   for i in range(2)]
            for t_ in hA + hB2 + cmb:
                nc.vector.memset(t_[:], 0.0)

            c_s = cpool.tile([64, 47 * 64], bf16)

            def load_x(t):
                # x tiles for step t: TA (4 blocks), TB (4 blocks), cmb
                # x-block, issued one step ahead. Loads shrink with the halo:
                # step t only reads rows 0..R-1 of the padded image.
                xlen = (48 - t) * WP - 2
                xa = xpool.tile([128, FLAT], bf16, tag="xa", name="xa")
                xb = xpool.tile([128, FLAT], bf16, tag="xb", name="xb")
                for b3, (dy, dx) in enumerate(X_TAPS_A):
                    s = dy * WP + dx
                    nc.gpsimd.dma_start(xa[32 * b3:32 * b3 + 32, 0:xlen],
                                        xp_d[t - 1, :, s:s + xlen])
                for b3, (dy, dx) in enumerate(X_TAPS_B):
                    s = dy * WP + dx
                    nc.sync.dma_start(xb[32 * b3:32 * b3 + 32, 0:xlen],
                                      xp_d[t - 1, :, s:s + xlen])
                nc.gpsimd.dma_start(cmb[t % 2][0:32, 0:xlen],
                                    xp_d[t - 1, :, 134:134 + xlen])
                return xa, xb

            xtiles = load_x(1)
            for t in range(1, T + 1):
                R = 48 - t
                xa, xb = xtiles
                if t < T:
                    xtiles = load_x(t + 1)
                hAp, hB2p, cmbp = hA[(t - 1) % 2], hB2[(t - 1) % 2], cmb[t % 2]
                hAc, hB2c, cmbn = hA[t % 2], hB2[t % 2], cmb[(t + 1) % 2]
                xav = xa[:].rearrange("p (y x) -> p y x", x=WP)
                xbv = xb[:].rearrange("p (y x) -> p y x", x=WP)
                hAv = hAp[:].rearrange("p (y x) -> p y x", x=WP)
                hB2v = hB2p[:].rearrange("p (y x) -> p y x", x=WP)
                cmbv = cmbp[:].rearrange("p (y x) -> p y x", x=WP)

                # 8-row chunks, one 1-bank PSUM tile pair each; bufs=4 keeps
                # four chunks in flight for deep PE pipelining
                chunks = [(y0, min(8, R - y0)) for y0 in range(0, R, 8)]
                hw3 = hAc[64:128, :].rearrange("p (y x) -> p y x", x=WP)

                for q, (y0, nr) in enumerate(chunks):
                    N = nr * 64
                    psA = psum.tile([128, 512], f32, tag="psA", name="psA")
                    psB = psum.tile([128, 512], f32, tag="psB", name="psB")
                    for gp, pt in ((0, psA), (1, psB)):
                        gs = gp * 128
                        dst = pt[:, 0:N]
                        nc.tensor.matmul(dst, wx1_s[:, gs:gs + 128],
                                         xav[:, y0:y0 + nr, 0:64],
                                         start=True, stop=False)
                        nc.tensor.matmul(dst, wx2_s[:, gs:gs + 128],
                                         xbv[:, y0:y0 + nr, 0:64],
                                         start=False, stop=False)
                    for gp, pt in ((0, psA), (1, psB)):
                        gs = gp * 128
                        dst = pt[:, 0:N]
                        if t > 1:
                            for dy in range(3):
                                nc.tensor.matmul(
                                    dst,
                                    wa_s[:, (dy * 2 + gp) * 128:
                                         (dy * 2 + gp + 1) * 128],
                                    hAv[:, y0 + dy:y0 + dy + nr, 0:64],
                                    start=False, stop=False)
                            nc.tensor.matmul(dst, wb2_s[:, gs:gs + 128],
                                             hB2v[:, y0:y0 + nr, 0:64],
                                             start=False, stop=False)
                            nc.tensor.matmul(dst, wc_s[:, gs:gs + 128],
                                             cmbv[0:96, y0:y0 + nr, 0:64],
                                             start=False, stop=True)
                        else:
                            nc.tensor.matmul(dst, wc_s[0:32, gs:gs + 128],
                                             cmbv[0:32, y0:y0 + nr, 0:64],
                                             start=False, stop=True)

                    # gating: 3 gate ACTs (sigma merged [f;o]; tanh(g) needs
                    # no partition shift with the [g;i] grouping; sigma(i)
                    # shifts down), then the c chain on DVE, tanh(c) shifted
                    # up, and h = o*tanh(c) written straight into the padded
                    # hA image (parts 64:128 = dx=1 col flavor; pad cols
                    # stay zero from init). Every other flavor is then a
                    # CONTIGUOUS whole-row copy of this region: one ~2KB
                    # descriptor per partition instead of 8 x 128B.
                    s01 = spool.tile([128, 512], bf16, tag="s01", name="s01")
                    s23 = spool.tile([128, 512], bf16, tag="s23", name="s23")
                    gt = spool.tile([64, 512], bf16, tag="gt", name="gt")
                    tc_ = spool.tile([128, 512], bf16, tag="tc", name="tc")
                    t1 = spool.tile([64, 512], bf16, tag="t1", name="t1")
                    c_sl = c_s[:, y0 * 64:y0 * 64 + N]
                    # one sigma covers i AND g via tanh(x) = 2*sigmoid(2x)-1
                    # (g weights+bias doubled on the host); the 2s-1 fixup
                    # runs on the DVE, partition-shifting P1->P0
                    nc.scalar.activation(s01[:, :N], psA[:, :N], Sigmoid,
                                         bias=bias_s[:, 0:1])
                    nc.scalar.activation(s23[:, :N], psB[:, :N], Sigmoid,
                                         bias=bias_s[:, 1:2])
                    nc.vector.tensor_scalar(gt[:, :N], s01[64:128, :N],
                                            2.0, -1.0, op0=mult, op1=add)
                    if t == 1:
                        nc.vector.tensor_mul(c_sl, s01[0:64, :N], gt[:, :N])
                    else:
                        nc.vector.tensor_mul(c_sl, s23[0:64, :N], c_sl)
                        nc.vector.tensor_mul(t1[:, :N], s01[0:64, :N],
                                             gt[:, :N])
                        nc.vector.tensor_add(c_sl, c_sl, t1[:, :N])
                    nc.scalar.activation(tc_[64:128, :N], c_sl, Tanh)
                    nc.vector.tensor_mul(
                        hw3[:, y0 + 1:y0 + 1 + nr, 1:65],
                        s23[64:128, :N].rearrange("p (y x) -> p y x", x=64),
                        tc_[64:128, :N].rearrange("p (y x) -> p y x", x=64))
                    d0 = (y0 + 1) * WP
                    L = nr * WP
                    if t < T:
                        # dx=0 flavor: shift by one column
                        nc.sync.dma_start(hAc[0:64, d0:d0 + L],
                                          hAc[64:128, d0 + 1:d0 + 1 + L])
                        # dx=2 flavor at row offsets 0 / 2 / 1
                        nc.sync.dma_start(hB2c[0:64, d0:d0 + L],
                                          hAc[64:128, d0 + 2:d0 + 2 + L])
                        if q == 0:
                            nc.gpsimd.dma_start(
                                hB2c[64:128, 0:L - WP],
                                hAc[64:128, d0 + WP + 2:d0 + 2 + L])
                        else:
                            nc.gpsimd.dma_start(
                                hB2c[64:128, d0 - 2 * WP:d0 - 2 * WP + L],
                                hAc[64:128, d0 + 2:d0 + 2 + L])
                        nc.gpsimd.dma_start(
                            cmbn[32:96, d0 - WP:d0 - WP + L],
                            hAc[64:128, d0 + 2:d0 + 2 + L])
                    if y0 < 32:
                        o0 = y0 * WP
                        nc.sync.dma_start(
                            out_d[t - 1, :, o0:o0 + L],
                            hAc[64:128, d0:d0 + L])

    nc.compile()
    return nc


def _host_prep(x, w_x2h, b_x2h, w_h2h, b_h2h):
    """Build the 8 per-core input maps."""
    import ml_dtypes
    np_bf16 = ml_dtypes.bfloat16

    x = np.ascontiguousarray(np.asarray(x, np.float32))
    w_x2h = np.asarray(w_x2h, np.float32)
    b_x2h = np.asarray(b_x2h, np.float32)
    w_h2h = np.asarray(w_h2h, np.float32)
    b_h2h = np.asarray(b_h2h, np.float32)

    bsum = b_x2h + b_h2h
    # gate-pair grouping: gp0 = [g; i], gp1 = [f; o]
    bias = np.zeros((128, 2), np.float32)
    bias[0:64, 0] = bsum[0:64]
    bias[64:128, 0] = 2.0 * bsum[128:192]
    bias[0:64, 1] = bsum[64:128]
    bias[64:128, 1] = bsum[192:256]

    # packed out-channel index list per gate-pair: gp0 = [i; 2g], gp1 = [f; o]
    oc_idx = {0: np.r_[0:64, 128:192], 1: np.r_[64:128, 192:256]}
    oc_scale = {0: np.r_[np.ones(64), 2.0 * np.ones(64)].astype(np.float32),
                1: np.ones(128, np.float32)}

    in_maps = []
    packed_w = {}
    for parity in range(2):
        wx_f = w_x2h if parity == 0 else w_x2h[:, :, ::-1, :]
        wh_f = w_h2h if parity == 0 else w_h2h[:, :, ::-1, :]
        wx1 = np.zeros((128, 2, 128), np.float32)
        wx2 = np.zeros((128, 2, 128), np.float32)
        wa = np.zeros((128, 3, 2, 128), np.float32)
        wb2 = np.zeros((128, 2, 128), np.float32)
        wc = np.zeros((96, 2, 128), np.float32)
        for gp in range(2):
            oc, sc = oc_idx[gp], oc_scale[gp][None, :]
            for b3, (dy, dx) in enumerate(X_TAPS_A):
                wx1[32 * b3:32 * b3 + 32, gp, :] = wx_f[oc, :, dy, dx].T * sc
            for b3, (dy, dx) in enumerate(X_TAPS_B):
                wx2[32 * b3:32 * b3 + 32, gp, :] = wx_f[oc, :, dy, dx].T * sc
            for dy in range(3):
                for b3 in range(2):
                    # parts 0:64 = dx1 flavor, 64:128 = dx0 (TT writes dx1)
                    wa[64 * b3:64 * b3 + 64, dy, gp, :] = \
                        wh_f[oc, :, dy, 1 - b3].T * sc
            wb2[0:64, gp, :] = wh_f[oc, :, 0, 2].T * sc
            wb2[64:128, gp, :] = wh_f[oc, :, 2, 2].T * sc
            wc[0:32, gp, :] = wx_f[oc, :, 2, 2].T * sc
            wc[32:96, gp, :] = wh_f[oc, :, 1, 2].T * sc
        packed_w[parity] = tuple(
            np.ascontiguousarray(a.reshape(a.shape[0], -1).astype(np_bf16))
            for a in (wx1, wx2, wa, wb2, wc))

    for core in range(8):
        b, parity = core // 2, core % 2
        xv = x[:, b]
        if parity == 1:
            xv = xv[:, :, ::-1, :]
        xp = np.zeros((T, C, HP, WP), np.float32)
        xp[:, :, 1:49, 1:65] = xv[:, :, 0:48, :]
        wx1, wx2, wa, wb2, wc = packed_w[parity]
        in_maps.append({
            "xp": np.ascontiguousarray(xp.reshape(T, C, FLAT).astype(np_bf16)),
            "wx1": wx1, "wx2": wx2, "wa": wa, "wb2": wb2, "wc": wc,
            "bias": bias,
        })
    return in_maps


def kernel(x, w_x2h, b_x2h, w_h2h, b_h2h, _trace=False, _tmpdir=None):
    from concourse.bass_utils import run_bass_kernel_spmd

    if "nc" not in _CACHE:
        _CACHE["nc"] = _build_program()
    nc = _CACHE["nc"]

    in_maps = _host_prep(x, w_x2h, b_x2h, w_h2h, b_h2h)
    kw = {}
    if _trace:
        kw = dict(trace=True, tmpdir=_tmpdir)
    res = run_bass_kernel_spmd(nc, in_maps, core_ids=list(range(8)), **kw)

    full = np.zeros((T, B, HID, H, W), np.float32)
    for core in range(8):
        b, parity = core // 2, core % 2
        out = np.asarray(res.results[core]["out"]).astype(np.float32)
        out = out.reshape(T, HID, 32, 66)[:, :, :, 1:65]
        if parity == 0:
            full[:, b, :, 0:32] = out
        else:
            full[:, b, :, 32:64] = out[:, :, ::-1, :]
    if _trace:
        return full, res
    return full


# revision 50
# speedup vs baseline: 1.1822x; 1.0010x over previous
"""ConvLSTM (T=16, B=4, C=32, HID=64, 64x64, 3x3 convs) on 8 Trainium2 cores.

Decomposition: 8 cores = batch(4) x H-halves(2). Each core owns 32 output rows
and recomputes a shrinking halo (rows 0..47-t at step t) so NO inter-core
communication is needed. The bottom-half cores get vertically flipped inputs
(and dy-flipped conv weights) so all 8 cores run the identical SPMD program.

Conv-as-matmul with dense tap packing (7 matmuls per 128-out-channel group per
512-pixel chunk = the partition-packing floor for 864 contraction rows):
    X1 = x taps (0,0)(0,1)(0,2)(1,0)   [4x32 rows, shifts 0,1,2,66]
    X2 = x taps (1,1)(1,2)(2,0)(2,1)   [shifts 67,68,132,133]
    HA @ dy, dy=0,1,2 = h taps (dy,0)(dy,1)  [2x64 rows, col flavors -1,0]
    HB2 = h taps (0,2)(2,2)            [col flavor +1 at row offsets 0,2]
    CMB = x tap (2,2) + h tap (1,2)    [96 rows]

This revision restructures the dataflow around measured per-instruction
fixed costs (DVE ~151 cyc, ACT ~352 cyc, PE p-state ramp 0.65->2.4 GHz,
~100 ns semaphore hops) and DMA packet-rate limits:
  * h = o*tanh(c) is written by the DVE straight into the padded hA image
    (parts 64:128 = the dx=1 column flavor; pad columns stay zero from the
    init memset). Every other tap flavor (dx=0 via hA[0:64], dx=2 via hB2
    and cmb) is then ONE contiguous whole-row SBUF copy at a flat offset:
    64 x ~1KB descriptors instead of 512 x 128B - this removed the ~200k
    DMA-packet storm that dominated the original kernel.
  * PSUM gate grouping [i; 2g] / [f; o] with g weights+bias doubled: each
    gate pair is ONE 128-partition Sigmoid ACT; tanh(g) = 2*sigmoid(2g)-1
    is reconstructed by a DVE tensor_scalar (which, unlike TensorTensor,
    may shift partition bases P1->P0); tanh(c) shifts up via ACT so every
    DVE product pairs at a single partition base.
  * 8-row chunks with 1-bank PSUM tiles, bufs=4: four chunks in flight
    keep the PE streaming (p-state at full clock); x-only matmuls are
    emitted before h matmuls per chunk for dependency-free boundary work.
  * All gating tensors and c in bf16 (DVE 2x mode).
  * DMA queues: sync = xb loads + dx0/dx2 flavor copies + output;
    gpsimd = xa loads + hB2[64:128]/cmb copies; scalar = ACTs only.
  * Output is written as padded 66-wide rows (contiguous) in bf16; the
    host strips pads and widens.
"""
import sys
import os

for _p in ("/opt/trn_rl_repo", "/root/.axon_site"):
    if _p not in sys.path and os.path.isdir(_p):
        sys.path.append(_p)

import numpy as np

T, B, C, H, W, HID = 16, 4, 32, 64, 64, 64
HP, WP = 49, 66          # padded per-core image: 48 data rows + 1 top pad, 64+2 cols
FLAT = HP * WP           # 3234
XLEN = 3100              # per-block x DMA length (covers max read f=3099)

X_TAPS_A = [(0, 0), (0, 1), (0, 2), (1, 0)]
X_TAPS_B = [(1, 1), (1, 2), (2, 0), (2, 1)]

_CACHE = {}


def _build_program():
    import concourse.mybir as mybir
    import concourse.tile as tile
    from concourse import bacc

    f32 = mybir.dt.float32
    bf16 = mybir.dt.bfloat16

    nc = bacc.Bacc("TRN2", target_bir_lowering=False, debug=False, num_devices=8)

    xp_d = nc.dram_tensor("xp", [T, C, FLAT], bf16, kind="ExternalInput")
    wx1_d = nc.dram_tensor("wx1", [128, 256], bf16, kind="ExternalInput")
    wx2_d = nc.dram_tensor("wx2", [128, 256], bf16, kind="ExternalInput")
    wa_d = nc.dram_tensor("wa", [128, 768], bf16, kind="ExternalInput")
    wb2_d = nc.dram_tensor("wb2", [128, 256], bf16, kind="ExternalInput")
    wc_d = nc.dram_tensor("wc", [96, 256], bf16, kind="ExternalInput")
    bias_d = nc.dram_tensor("bias", [128, 2], f32, kind="ExternalInput")
    # 2 blocks of 16 padded 66-wide rows per step; host strips the pads
    out_d = nc.dram_tensor("out", [T, HID, 2112], bf16, kind="ExternalOutput")

    Sigmoid = mybir.ActivationFunctionType.Sigmoid
    Tanh = mybir.ActivationFunctionType.Tanh
    mult = mybir.AluOpType.mult
    add = mybir.AluOpType.add

    with tile.TileContext(nc) as tc:
        with tc.tile_pool(name="const", bufs=1) as constp, \
             tc.tile_pool(name="xpool", bufs=3) as xpool, \
             tc.tile_pool(name="hpool", bufs=1) as hpool, \
             tc.tile_pool(name="cpool", bufs=1) as cpool, \
             tc.tile_pool(name="psum", bufs=4, space="PSUM") as psum, \
             tc.tile_pool(name="spool", bufs=3) as spool:

            wx1_s = constp.tile([128, 256], bf16)
            wx2_s = constp.tile([128, 256], bf16)
            wa_s = constp.tile([128, 768], bf16)
            wb2_s = constp.tile([128, 256], bf16)
            wc_s = constp.tile([96, 256], bf16)
            bias_s = constp.tile([128, 2], f32)
            for s_, d_ in [(wx1_s, wx1_d), (wx2_s, wx2_d), (wa_s, wa_d),
                           (wb2_s, wb2_d), (wc_s, wc_d), (bias_s, bias_d)]:
                nc.sync.dma_start(s_[:], d_[:])

            # ping-pong h tiles (shifted partition blocks, see module docstring)
            hA = [hpool.tile([128, FLAT], bf16, tag=f"hA{i}", name=f"hA{i}")
                  for i in range(2)]
            hB2 = [hpool.tile([128, FLAT], bf16, tag=f"hB2{i}", name=f"hB2{i}")
                   for i in range(2)]
            # cmb: parts 0-31 x tap (2,2) [per-step], parts 32-95 h tap (1,2)
            cmb = [hpool.tile([96, FLAT], bf16, tag=f"cmb{i}", name=f"cmb{i}")
                   for i in range(2)]
            for t_ in hA + hB2 + cmb:
                nc.vector.memset(t_[:], 0.0)

            c_s = cpool.tile([64, 47 * 64], bf16)

            def load_x(t):
                # x tiles for step t: TA (4 blocks), TB (4 blocks), cmb
                # x-block, issued one step ahead. Loads shrink with the halo:
                # step t only reads rows 0..R-1 of the padded image.
                xlen = (48 - t) * WP - 2
                xa = xpool.tile([128, FLAT], bf16, tag="xa", name="xa")
                xb = xpool.tile([128, FLAT], bf16, tag="xb", name="xb")
                for b3, (dy, dx) in enumerate(X_TAPS_A):
                    s = dy * WP + dx
                    nc.gpsimd.dma_start(xa[32 * b3:32 * b3 + 32, 0:xlen],
                                        xp_d[t - 1, :, s:s + xlen])
                for b3, (dy, dx) in enumerate(X_TAPS_B):
                    s = dy * WP + dx
                    nc.sync.dma_start(xb[32 * b3:32 * b3 + 32, 0:xlen],
                                      xp_d[t - 1, :, s:s + xlen])
                nc.gpsimd.dma_start(cmb[t % 2][0:32, 0:xlen],
                                    xp_d[t - 1, :, 134:134 + xlen])
                return xa, xb

            xtiles = load_x(1)
            for t in range(1, T + 1):
                R = 48 - t
                xa, xb = xtiles
                if t < T:
                    xtiles = load_x(t + 1)
                hAp, hB2p, cmbp = hA[(t - 1) % 2], hB2[(t - 1) % 2], cmb[t % 2]
                hAc, hB2c, cmbn = hA[t % 2], hB2[t % 2], cmb[(t + 1) % 2]
                xav = xa[:].rearrange("p (y x) -> p y x", x=WP)
                xbv = xb[:].rearrange("p (y x) -> p y x", x=WP)
                hAv = hAp[:].rearrange("p (y x) -> p y x", x=WP)
                hB2v = hB2p[:].rearrange("p (y x) -> p y x", x=WP)
                cmbv = cmbp[:].rearrange("p (y x) -> p y x", x=WP)

                # 8-row chunks, one 1-bank PSUM tile pair each; bufs=4 keeps
                # four chunks in flight for deep PE pipelining
                chunks = [(y0, min(8, R - y0)) for y0 in range(0, R, 8)]
                hw3 = hAc[64:128, :].rearrange("p (y x) -> p y x", x=WP)

                for q, (y0, nr) in enumerate(chunks):
                    N = nr * 64
                    psA = psum.tile([128, 512], f32, tag="psA", name="psA")
                    psB = psum.tile([128, 512], f32, tag="psB", name="psB")
                    for gp, pt in ((0, psA), (1, psB)):
                        gs = gp * 128
                        dst = pt[:, 0:N]
                        nc.tensor.matmul(dst, wx1_s[:, gs:gs + 128],
                                         xav[:, y0:y0 + nr, 0:64],
                                         start=True, stop=False)
                        nc.tensor.matmul(dst, wx2_s[:, gs:gs + 128],
                                         xbv[:, y0:y0 + nr, 0:64],
                                         start=False, stop=False)
                    for gp, pt in ((0, psA), (1, psB)):
                        gs = gp * 128
                        dst = pt[:, 0:N]
                        if t > 1:
                            for dy in range(3):
                                nc.tensor.matmul(
                                    dst,
                                    wa_s[:, (dy * 2 + gp) * 128:
                                         (dy * 2 + gp + 1) * 128],
                                    hAv[:, y0 + dy:y0 + dy + nr, 0:64],
                                    start=False, stop=False)
                            nc.tensor.matmul(dst, wb2_s[:, gs:gs + 128],
                                             hB2v[:, y0:y0 + nr, 0:64],
                                             start=False, stop=False)
                            nc.tensor.matmul(dst, wc_s[:, gs:gs + 128],
                                             cmbv[0:96, y0:y0 + nr, 0:64],
                                             start=False, stop=True)
                        else:
                            nc.tensor.matmul(dst, wc_s[0:32, gs:gs + 128],
                                             cmbv[0:32, y0:y0 + nr, 0:64],
                                             start=False, stop=True)

                    # gating: 3 gate ACTs (sigma merged [f;o]; tanh(g) needs
                    # no partition shift with the [g;i] grouping; sigma(i)
                    # shifts down), then the c chain on DVE, tanh(c) shifted
                    # up, and h = o*tanh(c) written straight into the padded
                    # hA image (parts 64:128 = dx=1 col flavor; pad cols
                    # stay zero from init). Every other flavor is then a
                    # CONTIGUOUS whole-row copy of this region: one ~2KB
                    # descriptor per partition instead of 8 x 128B.
                    s01 = spool.tile([128, 512], bf16, tag="s01", name="s01")
                    s23 = spool.tile([128, 512], bf16, tag="s23", name="s23")
                    gt = spool.tile([64, 512], bf16, tag="gt", name="gt")
                    tc_ = spool.tile([128, 512], bf16, tag="tc", name="tc")
                    t1 = spool.tile([64, 512], bf16, tag="t1", name="t1")
                    c_sl = c_s[:, y0 * 64:y0 * 64 + N]
                    # one sigma covers i AND g via tanh(x) = 2*sigmoid(2x)-1
                    # (g weights+bias doubled on the host); the 2s-1 fixup
                    # runs on the DVE, partition-shifting P1->P0
                    nc.scalar.activation(s01[:, :N], psA[:, :N], Sigmoid,
                                         bias=bias_s[:, 0:1])
                    nc.scalar.activation(s23[:, :N], psB[:, :N], Sigmoid,
                                         bias=bias_s[:, 1:2])
                    nc.vector.tensor_scalar(gt[:, :N], s01[64:128, :N],
                                            2.0, -1.0, op0=mult, op1=add)
                    if t == 1:
                        nc.vector.tensor_mul(c_sl, s01[0:64, :N], gt[:, :N])
                    else:
                        nc.vector.tensor_mul(c_sl, s23[0:64, :N], c_sl)
                        nc.vector.tensor_mul(t1[:, :N], s01[0:64, :N],
                                             gt[:, :N])
                        nc.vector.tensor_add(c_sl, c_sl, t1[:, :N])
                    nc.scalar.activation(tc_[64:128, :N], c_sl, Tanh)
                    nc.vector.tensor_mul(
                        hw3[:, y0 + 1:y0 + 1 + nr, 1:65],
                        s23[64:128, :N].rearrange("p (y x) -> p y x", x=64),
                        tc_[64:128, :N].rearrange("p (y x) -> p y x", x=64))
                    d0 = (y0 + 1) * WP
                    L = nr * WP
                    if t < T:
                        # dx=0 flavor: shift by one column
                        nc.sync.dma_start(hAc[0:64, d0:d0 + L],
                                          hAc[64:128, d0 + 1:d0 + 1 + L])
                        # dx=2 flavor at row offsets 0 / 2 / 1
                        nc.sync.dma_start(hB2c[0:64, d0:d0 + L],
                                          hAc[64:128, d0 + 2:d0 + 2 + L])
                        if q == 0:
                            nc.gpsimd.dma_start(
                                hB2c[64:128, 0:L - WP],
                                hAc[64:128, d0 + WP + 2:d0 + 2 + L])
                        else:
                            nc.gpsimd.dma_start(
                                hB2c[64:128, d0 - 2 * WP:d0 - 2 * WP + L],
                                hAc[64:128, d0 + 2:d0 + 2 + L])
                        nc.gpsimd.dma_start(
                            cmbn[32:96, d0 - WP:d0 - WP + L],
                            hAc[64:128, d0 + 2:d0 + 2 + L])
                    if y0 < 32:
                        o0 = y0 * WP
                        nc.sync.dma_start(
                            out_d[t - 1, :, o0:o0 + L],
                            hAc[64:128, d0:d0 + L])

    nc.compile()
    return nc


def _host_prep(x, w_x2h, b_x2h, w_h2h, b_h2h):
    """Build the 8 per-core input maps."""
    import ml_dtypes
    np_bf16 = ml_dtypes.bfloat16

    x = np.ascontiguousarray(np.asarray(x, np.float32))
    w_x2h = np.asarray(w_x2h, np.float32)
    b_x2h = np.asarray(b_x2h, np.float32)
    w_h2h = np.asarray(w_h2h, np.float32)
    b_h2h = np.asarray(b_h2h, np.float32)

    bsum = b_x2h + b_h2h
    # gate-pair grouping: gp0 = [g; i], gp1 = [f; o]
    bias = np.zeros((128, 2), np.float32)
    bias[0:64, 0] = bsum[0:64]
    bias[64:128, 0] = 2.0 * bsum[128:192]
    bias[0:64, 1] = bsum[64:128]
    bias[64:128, 1] = bsum[192:256]

    # packed out-channel index list per gate-pair: gp0 = [i; 2g], gp1 = [f; o]
    oc_idx = {0: np.r_[0:64, 128:192], 1: np.r_[64:128, 192:256]}
    oc_scale = {0: np.r_[np.ones(64), 2.0 * np.ones(64)].astype(np.float32),
                1: np.ones(128, np.float32)}

    in_maps = []
    packed_w = {}
    for parity in range(2):
        wx_f = w_x2h if parity == 0 else w_x2h[:, :, ::-1, :]
        wh_f = w_h2h if parity == 0 else w_h2h[:, :, ::-1, :]
        wx1 = np.zeros((128, 2, 128), np.float32)
        wx2 = np.zeros((128, 2, 128), np.float32)
        wa = np.zeros((128, 3, 2, 128), np.float32)
        wb2 = np.zeros((128, 2, 128), np.float32)
        wc = np.zeros((96, 2, 128), np.float32)
        for gp in range(2):
            oc, sc = oc_idx[gp], oc_scale[gp][None, :]
            for b3, (dy, dx) in enumerate(X_TAPS_A):
                wx1[32 * b3:32 * b3 + 32, gp, :] = wx_f[oc, :, dy, dx].T * sc
            for b3, (dy, dx) in enumerate(X_TAPS_B):
                wx2[32 * b3:32 * b3 + 32, gp, :] = wx_f[oc, :, dy, dx].T * sc
            for dy in range(3):
                for b3 in range(2):
                    # parts 0:64 = dx1 flavor, 64:128 = dx0 (TT writes dx1)
                    wa[64 * b3:64 * b3 + 64, dy, gp, :] = \
                        wh_f[oc, :, dy, 1 - b3].T * sc
            wb2[0:64, gp, :] = wh_f[oc, :, 0, 2].T * sc
            wb2[64:128, gp, :] = wh_f[oc, :, 2, 2].T * sc
            wc[0:32, gp, :] = wx_f[oc, :, 2, 2].T * sc
            wc[32:96, gp, :] = wh_f[oc, :, 1, 2].T * sc
        packed_w[parity] = tuple(
            np.ascontiguousarray(a.reshape(a.shape[0], -1).astype(np_bf16))
            for a in (wx1, wx2, wa, wb2, wc))

    for core in range(8):
        b, parity = core // 2, core % 2
        xv = x[:, b]
        if parity == 1:
            xv = xv[:, :, ::-1, :]
        xp = np.zeros((T, C, HP, WP), np.float32)
        xp[:, :, 1:49, 1:65] = xv[:, :, 0:48, :]
        wx1, wx2, wa, wb2, wc = packed_w[parity]
        in_maps.append({
            "xp": np.ascontiguousarray(xp.reshape(T, C, FLAT).astype(np_bf16)),
            "wx1": wx1, "wx2": wx2, "wa": wa, "wb2": wb2, "wc": wc,
            "bias": bias,
        })
    return in_maps


def kernel(x, w_x2h, b_x2h, w_h2h, b_h2h, _trace=False, _tmpdir=None):
    from concourse.bass_utils import run_bass_kernel_spmd

    if "nc" not in _CACHE:
        _CACHE["nc"] = _build_program()
    nc = _CACHE["nc"]

    in_maps = _host_prep(x, w_x2h, b_x2h, w_h2h, b_h2h)
    kw = {}
    if _trace:
        kw = dict(trace=True, tmpdir=_tmpdir)
    res = run_bass_kernel_spmd(nc, in_maps, core_ids=list(range(8)), **kw)

    full = np.zeros((T, B, HID, H, W), np.float32)
    for core in range(8):
        b, parity = core // 2, core % 2
        out = np.asarray(res.results[core]["out"]).astype(np.float32)
        out = out.reshape(T, HID, 32, 66)[:, :, :, 1:65]
        if parity == 0:
            full[:, b, :, 0:32] = out
        else:
            full[:, b, :, 32:64] = out[:, :, ::-1, :]
    if _trace:
        return full, res
    return full


# revision 52
# speedup vs baseline: 1.1893x; 1.0060x over previous
"""ConvLSTM (T=16, B=4, C=32, HID=64, 64x64, 3x3 convs) on 8 Trainium2 cores.

Decomposition: 8 cores = batch(4) x H-halves(2). Each core owns 32 output rows
and recomputes a shrinking halo (rows 0..47-t at step t) so NO inter-core
communication is needed. The bottom-half cores get vertically flipped inputs
(and dy-flipped conv weights) so all 8 cores run the identical SPMD program.

Conv-as-matmul with dense tap packing (7 matmuls per 128-out-channel group per
512-pixel chunk = the partition-packing floor for 864 contraction rows):
    X1 = x taps (0,0)(0,1)(0,2)(1,0)   [4x32 rows, shifts 0,1,2,66]
    X2 = x taps (1,1)(1,2)(2,0)(2,1)   [shifts 67,68,132,133]
    HA @ dy, dy=0,1,2 = h taps (dy,0)(dy,1)  [2x64 rows, col flavors -1,0]
    HB2 = h taps (0,2)(2,2)            [col flavor +1 at row offsets 0,2]
    CMB = x tap (2,2) + h tap (1,2)    [96 rows]

This revision restructures the dataflow around measured per-instruction
fixed costs (DVE ~151 cyc, ACT ~352 cyc, PE p-state ramp 0.65->2.4 GHz,
~100 ns semaphore hops) and DMA packet-rate limits:
  * h = o*tanh(c) is written by the DVE straight into the padded hA image
    (parts 64:128 = the dx=1 column flavor; pad columns stay zero from the
    init memset). Every other tap flavor (dx=0 via hA[0:64], dx=2 via hB2
    and cmb) is then ONE contiguous whole-row SBUF copy at a flat offset:
    64 x ~1KB descriptors instead of 512 x 128B - this removed the ~200k
    DMA-packet storm that dominated the original kernel.
  * PSUM gate grouping [i; 2g] / [f; o] with g weights+bias doubled: each
    gate pair is ONE 128-partition Sigmoid ACT; tanh(g) = 2*sigmoid(2g)-1
    is reconstructed by a DVE tensor_scalar (which, unlike TensorTensor,
    may shift partition bases P1->P0); tanh(c) shifts up via ACT so every
    DVE product pairs at a single partition base.
  * 8-row chunks with 1-bank PSUM tiles, bufs=4: four chunks in flight
    keep the PE streaming (p-state at full clock); x-only matmuls are
    emitted before h matmuls per chunk for dependency-free boundary work.
  * All gating tensors and c in bf16 (DVE 2x mode).
  * DMA queues: sync = xb loads + dx0/dx2 flavor copies + output;
    gpsimd = xa loads + hB2[64:128]/cmb copies; scalar = ACTs only.
  * Output is written as padded 66-wide rows (contiguous) in bf16; the
    host strips pads and widens.
"""
import sys
import os

for _p in ("/opt/trn_rl_repo", "/root/.axon_site"):
    if _p not in sys.path and os.path.isdir(_p):
        sys.path.append(_p)

import numpy as np

T, B, C, H, W, HID = 16, 4, 32, 64, 64, 64
HP, WP = 49, 66          # padded per-core image: 48 data rows + 1 top pad, 64+2 cols
FLAT = HP * WP           # 3234
XLEN = 3100              # per-block x DMA length (covers max read f=3099)

X_TAPS_A = [(0, 0), (0, 1), (0, 2), (1, 0)]
X_TAPS_B = [(1, 1), (1, 2), (2, 0), (2, 1)]

_CACHE = {}


def _build_program():
    import concourse.mybir as mybir
    import concourse.tile as tile
    from concourse import bacc

    f32 = mybir.dt.float32
    bf16 = mybir.dt.bfloat16

    nc = bacc.Bacc("TRN2", target_bir_lowering=False, debug=False, num_devices=8)

    xp_d = nc.dram_tensor("xp", [T, C, FLAT], bf16, kind="ExternalInput")
    wx1_d = nc.dram_tensor("wx1", [128, 256], bf16, kind="ExternalInput")
    wx2_d = nc.dram_tensor("wx2", [128, 256], bf16, kind="ExternalInput")
    wa_d = nc.dram_tensor("wa", [128, 768], bf16, kind="ExternalInput")
    wb2_d = nc.dram_tensor("wb2", [128, 256], bf16, kind="ExternalInput")
    wc_d = nc.dram_tensor("wc", [96, 256], bf16, kind="ExternalInput")
    bias_d = nc.dram_tensor("bias", [128, 2], f32, kind="ExternalInput")
    # 2 blocks of 16 padded 66-wide rows per step; host strips the pads
    out_d = nc.dram_tensor("out", [T, HID, 2112], bf16, kind="ExternalOutput")

    Sigmoid = mybir.ActivationFunctionType.Sigmoid
    Tanh = mybir.ActivationFunctionType.Tanh
    mult = mybir.AluOpType.mult
    add = mybir.AluOpType.add

    with tile.TileContext(nc) as tc:
        with tc.tile_pool(name="const", bufs=1) as constp, \
             tc.tile_pool(name="xpool", bufs=3) as xpool, \
             tc.tile_pool(name="hpool", bufs=1) as hpool, \
             tc.tile_pool(name="cpool", bufs=1) as cpool, \
             tc.tile_pool(name="psum", bufs=4, space="PSUM") as psum, \
             tc.tile_pool(name="spool", bufs=3) as spool:

            wx1_s = constp.tile([128, 256], bf16)
            wx2_s = constp.tile([128, 256], bf16)
            wa_s = constp.tile([128, 768], bf16)
            wb2_s = constp.tile([128, 256], bf16)
            wc_s = constp.tile([96, 256], bf16)
            bias_s = constp.tile([128, 2], f32)
            for s_, d_ in [(wx1_s, wx1_d), (wx2_s, wx2_d), (wa_s, wa_d),
                           (wb2_s, wb2_d), (wc_s, wc_d), (bias_s, bias_d)]:
                nc.sync.dma_start(s_[:], d_[:])

            # ping-pong h tiles (shifted partition blocks, see module docstring)
            hA = [hpool.tile([128, FLAT], bf16, tag=f"hA{i}", name=f"hA{i}")
                  for i in range(2)]
            hB2 = [hpool.tile([128, FLAT], bf16, tag=f"hB2{i}", name=f"hB2{i}")
                   for i in range(2)]
            # cmb: parts 0-31 x tap (2,2) [per-step], parts 32-95 h tap (1,2)
            cmb = [hpool.tile([96, FLAT], bf16, tag=f"cmb{i}", name=f"cmb{i}")
                   for i in range(2)]
            for t_ in hA + hB2 + cmb:
                nc.vector.memset(t_[:], 0.0)

            c_s = cpool.tile([64, 47 * 64], bf16)

            def load_x(t):
                # x tiles for step t: TA (4 blocks), TB (4 blocks), cmb
                # x-block, issued one step ahead. Loads shrink with the halo:
                # step t only reads rows 0..R-1 of the padded image.
                xlen = (48 - t) * WP - 2
                xa = xpool.tile([128, FLAT], bf16, tag="xa", name="xa")
                xb = xpool.tile([128, FLAT], bf16, tag="xb", name="xb")
                for b3, (dy, dx) in enumerate(X_TAPS_A):
                    s = dy * WP + dx
                    nc.gpsimd.dma_start(xa[32 * b3:32 * b3 + 32, 0:xlen],
                                        xp_d[t - 1, :, s:s + xlen])
                for b3, (dy, dx) in enumerate(X_TAPS_B):
                    s = dy * WP + dx
                    nc.sync.dma_start(xb[32 * b3:32 * b3 + 32, 0:xlen],
                                      xp_d[t - 1, :, s:s + xlen])
                nc.gpsimd.dma_start(cmb[t % 2][0:32, 0:xlen],
                                    xp_d[t - 1, :, 134:134 + xlen])
                return xa, xb

            xtiles = load_x(1)
            for t in range(1, T + 1):
                R = 48 - t
                xa, xb = xtiles
                if t < T:
                    xtiles = load_x(t + 1)
                hAp, hB2p, cmbp = hA[(t - 1) % 2], hB2[(t - 1) % 2], cmb[t % 2]
                hAc, hB2c, cmbn = hA[t % 2], hB2[t % 2], cmb[(t + 1) % 2]
                xav = xa[:].rearrange("p (y x) -> p y x", x=WP)
                xbv = xb[:].rearrange("p (y x) -> p y x", x=WP)
                hAv = hAp[:].rearrange("p (y x) -> p y x", x=WP)
                hB2v = hB2p[:].rearrange("p (y x) -> p y x", x=WP)
                cmbv = cmbp[:].rearrange("p (y x) -> p y x", x=WP)

                # 8-row chunks, one 1-bank PSUM tile pair each; bufs=4 keeps
                # four chunks in flight for deep PE pipelining
                chunks = [(y0, min(8, R - y0)) for y0 in range(0, R, 8)]
                hw3 = hAc[64:128, :].rearrange("p (y x) -> p y x", x=WP)

                def emit_tail(q, y0, nr, N, s23, c_sl):
                    tc_ = spool.tile([128, 512], bf16, tag="tc", name="tc")
                    nc.scalar.activation(tc_[64:128, :N], c_sl, Tanh)
                    nc.vector.tensor_mul(
                        hw3[:, y0 + 1:y0 + 1 + nr, 1:65],
                        s23[64:128, :N].rearrange("p (y x) -> p y x", x=64),
                        tc_[64:128, :N].rearrange("p (y x) -> p y x", x=64))
                    d0 = (y0 + 1) * WP
                    L = nr * WP
                    if t < T:
                        # dx=0 flavor: shift by one column
                        nc.sync.dma_start(hAc[0:64, d0:d0 + L],
                                          hAc[64:128, d0 + 1:d0 + 1 + L])
                        # dx=2 flavor at row offsets 0 / 2 / 1
                        nc.sync.dma_start(hB2c[0:64, d0:d0 + L],
                                          hAc[64:128, d0 + 2:d0 + 2 + L])
                        if q == 0:
                            nc.gpsimd.dma_start(
                                hB2c[64:128, 0:L - WP],
                                hAc[64:128, d0 + WP + 2:d0 + 2 + L])
                        else:
                            nc.gpsimd.dma_start(
                                hB2c[64:128, d0 - 2 * WP:d0 - 2 * WP + L],
                                hAc[64:128, d0 + 2:d0 + 2 + L])
                        nc.gpsimd.dma_start(
                            cmbn[32:96, d0 - WP:d0 - WP + L],
                            hAc[64:128, d0 + 2:d0 + 2 + L])
                    if y0 < 32:
                        o0 = y0 * WP
                        nc.sync.dma_start(
                            out_d[t - 1, :, o0:o0 + L],
                            hAc[64:128, d0:d0 + L])

                tail = None
                for q, (y0, nr) in enumerate(chunks):
                    N = nr * 64
                    psA = psum.tile([128, 512], f32, tag="psA", name="psA")
                    psB = psum.tile([128, 512], f32, tag="psB", name="psB")
                    for gp, pt in ((0, psA), (1, psB)):
                        gs = gp * 128
                        dst = pt[:, 0:N]
                        nc.tensor.matmul(dst, wx1_s[:, gs:gs + 128],
                                         xav[:, y0:y0 + nr, 0:64],
                                         start=True, stop=False)
                        nc.tensor.matmul(dst, wx2_s[:, gs:gs + 128],
                                         xbv[:, y0:y0 + nr, 0:64],
                                         start=False, stop=False)
                    for gp, pt in ((0, psA), (1, psB)):
                        gs = gp * 128
                        dst = pt[:, 0:N]
                        if t > 1:
                            for dy in range(3):
                                nc.tensor.matmul(
                                    dst,
                                    wa_s[:, (dy * 2 + gp) * 128:
                                         (dy * 2 + gp + 1) * 128],
                                    hAv[:, y0 + dy:y0 + dy + nr, 0:64],
                                    start=False, stop=False)
                            nc.tensor.matmul(dst, wb2_s[:, gs:gs + 128],
                                             hB2v[:, y0:y0 + nr, 0:64],
                                             start=False, stop=False)
                            nc.tensor.matmul(dst, wc_s[:, gs:gs + 128],
                                             cmbv[0:96, y0:y0 + nr, 0:64],
                                             start=False, stop=True)
                        else:
                            nc.tensor.matmul(dst, wc_s[0:32, gs:gs + 128],
                                             cmbv[0:32, y0:y0 + nr, 0:64],
                                             start=False, stop=True)

                    # gating: 3 gate ACTs (sigma merged [f;o]; tanh(g) needs
                    # no partition shift with the [g;i] grouping; sigma(i)
                    # shifts down), then the c chain on DVE, tanh(c) shifted
                    # up, and h = o*tanh(c) written straight into the padded
                    # hA image (parts 64:128 = dx=1 col flavor; pad cols
                    # stay zero from init). Every other flavor is then a
                    # CONTIGUOUS whole-row copy of this region: one ~2KB
                    # descriptor per partition instead of 8 x 128B.
                    s01 = spool.tile([128, 512], bf16, tag="s01", name="s01")
                    s23 = spool.tile([128, 512], bf16, tag="s23", name="s23")
                    gt = spool.tile([64, 512], bf16, tag="gt", name="gt")
                    t1 = spool.tile([64, 512], bf16, tag="t1", name="t1")
                    c_sl = c_s[:, y0 * 64:y0 * 64 + N]
                    # one sigma covers i AND g via tanh(x) = 2*sigmoid(2x)-1
                    # (g weights+bias doubled on the host); the 2s-1 fixup
                    # runs on the DVE, partition-shifting P1->P0
                    nc.scalar.activation(s01[:, :N], psA[:, :N], Sigmoid,
                                         bias=bias_s[:, 0:1])
                    nc.scalar.activation(s23[:, :N], psB[:, :N], Sigmoid,
                                         bias=bias_s[:, 1:2])
                    nc.vector.tensor_scalar(gt[:, :N], s01[64:128, :N],
                                            2.0, -1.0, op0=mult, op1=add)
                    if t == 1:
                        nc.vector.tensor_mul(c_sl, s01[0:64, :N], gt[:, :N])
                    else:
                        nc.vector.tensor_mul(c_sl, s23[0:64, :N], c_sl)
                        nc.vector.tensor_mul(t1[:, :N], s01[0:64, :N],
                                             gt[:, :N])
                        nc.vector.tensor_add(c_sl, c_sl, t1[:, :N])
                    # the tail (tanh(c), h-write, flavor copies, output) is
                    # emitted one chunk LATE: scalar runs the next chunk's
                    # sigmas instead of head-of-line-blocking on tanh(c)
                    # waiting for the DVE c chain
                    if tail is not None:
                        emit_tail(*tail)
                    tail = (q, y0, nr, N, s23, c_sl)
                if tail is not None:
                    emit_tail(*tail)

    nc.compile()
    return nc


def _host_prep(x, w_x2h, b_x2h, w_h2h, b_h2h):
    """Build the 8 per-core input maps."""
    import ml_dtypes
    np_bf16 = ml_dtypes.bfloat16

    x = np.ascontiguousarray(np.asarray(x, np.float32))
    w_x2h = np.asarray(w_x2h, np.float32)
    b_x2h = np.asarray(b_x2h, np.float32)
    w_h2h = np.asarray(w_h2h, np.float32)
    b_h2h = np.asarray(b_h2h, np.float32)

    bsum = b_x2h + b_h2h
    # gate-pair grouping: gp0 = [g; i], gp1 = [f; o]
    bias = np.zeros((128, 2), np.float32)
    bias[0:64, 0] = bsum[0:64]
    bias[64:128, 0] = 2.0 * bsum[128:192]
    bias[0:64, 1] = bsum[64:128]
    bias[64:128, 1] = bsum[192:256]

    # packed out-channel index list per gate-pair: gp0 = [i; 2g], gp1 = [f; o]
    oc_idx = {0: np.r_[0:64, 128:192], 1: np.r_[64:128, 192:256]}
    oc_scale = {0: np.r_[np.ones(64), 2.0 * np.ones(64)].astype(np.float32),
                1: np.ones(128, np.float32)}

    in_maps = []
    packed_w = {}
    for parity in range(2):
        wx_f = w_x2h if parity == 0 else w_x2h[:, :, ::-1, :]
        wh_f = w_h2h if parity == 0 else w_h2h[:, :, ::-1, :]
        wx1 = np.zeros((128, 2, 128), np.float32)
        wx2 = np.zeros((128, 2, 128), np.float32)
        wa = np.zeros((128, 3, 2, 128), np.float32)
        wb2 = np.zeros((128, 2, 128), np.float32)
        wc = np.zeros((96, 2, 128), np.float32)
        for gp in range(2):
            oc, sc = oc_idx[gp], oc_scale[gp][None, :]
            for b3, (dy, dx) in enumerate(X_TAPS_A):
                wx1[32 * b3:32 * b3 + 32, gp, :] = wx_f[oc, :, dy, dx].T * sc
            for b3, (dy, dx) in enumerate(X_TAPS_B):
                wx2[32 * b3:32 * b3 + 32, gp, :] = wx_f[oc, :, dy, dx].T * sc
            for dy in range(3):
                for b3 in range(2):
                    # parts 0:64 = dx1 flavor, 64:128 = dx0 (TT writes dx1)
                    wa[64 * b3:64 * b3 + 64, dy, gp, :] = \
                        wh_f[oc, :, dy, 1 - b3].T * sc
            wb2[0:64, gp, :] = wh_f[oc, :, 0, 2].T * sc
            wb2[64:128, gp, :] = wh_f[oc, :, 2, 2].T * sc
            wc[0:32, gp, :] = wx_f[oc, :, 2, 2].T * sc
            wc[32:96, gp, :] = wh_f[oc, :, 1, 2].T * sc
        packed_w[parity] = tuple(
            np.ascontiguousarray(a.reshape(a.shape[0], -1).astype(np_bf16))
            for a in (wx1, wx2, wa, wb2, wc))

    for core in range(8):
        b, parity = core // 2, core % 2
        xv = x[:, b]
        if parity == 1:
            xv = xv[:, :, ::-1, :]
        xp = np.zeros((T, C, HP, WP), np.float32)
        xp[:, :, 1:49, 1:65] = xv[:, :, 0:48, :]
        wx1, wx2, wa, wb2, wc = packed_w[parity]
        in_maps.append({
            "xp": np.ascontiguousarray(xp.reshape(T, C, FLAT).astype(np_bf16)),
            "wx1": wx1, "wx2": wx2, "wa": wa, "wb2": wb2, "wc": wc,
            "bias": bias,
        })
    return in_maps


def kernel(x, w_x2h, b_x2h, w_h2h, b_h2h, _trace=False, _tmpdir=None):
    from concourse.bass_utils import run_bass_kernel_spmd

    if "nc" not in _CACHE:
        _CACHE["nc"] = _build_program()
    nc = _CACHE["nc"]

    in_maps = _host_prep(x, w_x2h, b_x2h, w_h2h, b_h2h)
    kw = {}
    if _trace:
        kw = dict(trace=True, tmpdir=_tmpdir)
    res = run_bass_kernel_spmd(nc, in_maps, core_ids=list(range(8)), **kw)

    full = np.zeros((T, B, HID, H, W), np.float32)
    for core in range(8):
        b, parity = core // 2, core % 2
        out = np.asarray(res.results[core]["out"]).astype(np.float32)
        out = out.reshape(T, HID, 32, 66)[:, :, :, 1:65]
        if parity == 0:
            full[:, b, :, 0:32] = out
        else:
            full[:, b, :, 32:64] = out[:, :, ::-1, :]
    if _trace:
        return full, res
    return full
